# revision 46
# baseline (speedup 1.0000x reference)
"""Trainium2 Bass kernel for nn_DifferentialFlashAttention.

Computation (per token t, fully parallel over the B*N = 32768 tokens):
  qkv = x @ W_attn.T ; split into q, k, v
  q/k split per head into two sub-vectors (q1, q2 / k1, k2) of 32 dims
  S_s[i,j] = q_s[i] . k_s[j] / sqrt(32)   (attention over the 12 HEADS)
  A_s = softmax_j(S_s); O_s[i] = sum_j A_s[i,j] * v[j]
  y = (O_1 - lam_full * O_2) * (1 - LAMBDA_INIT);  out = y @ W_out.T

Sharding: data-parallel over tokens across 8 NeuronCores (4096 tokens each).
Per core: tokens-on-partitions layout, 32 tiles of 128 tokens.
  - GEMM1/GEMM2 on TensorE (stationary = transposed activations chunks,
    moving = pre-transposed weights held in SBUF).
  - The per-token head-attention on VectorE (broadcast-AP products +
    segmented reduces) and ScalarE (exp, PSUM->SBUF copies).
"""

import math
import sys

import numpy as np

if "/opt/trn_rl_repo" not in sys.path:
    sys.path.insert(0, "/opt/trn_rl_repo")

N_HEAD = 12
N_EMBD = 768
HEAD_DIM = 32  # per-sub head dim
DEPTH = 12
LAMBDA_INIT = 0.8 - 0.6 * math.exp(-0.3 * DEPTH)
B, N, D = 4, 8192, 768
N_CORES = 8
TOK_TOTAL = B * N
TOK_PER_CORE = TOK_TOTAL // N_CORES  # 4096
TILE_T = 128
KC = D // 128  # 6 contraction chunks
SCALE = 1.0 / math.sqrt(HEAD_DIM)

_PROG_CACHE = {}


def _build_program(
    ntok,
    nrep=1,
    parts=("gemm1", "attn", "tr", "gemm2"),
    gemm_f32r=False,
    attn_fp16=False,
    attn_s16=False,
    gpsimd_split=False,
):
    GPSIMD_SPLIT = gpsimd_split
    import concourse.bacc as bacc
    import concourse.tile as tile
    from concourse import mybir
    from concourse.masks import make_identity

    f32 = mybir.dt.float32
    f32r = mybir.dt.float32r
    fp16 = mybir.dt.float16
    Exp = mybir.ActivationFunctionType.Exp
    X = mybir.AxisListType.X
    add = mybir.AluOpType.add
    mult = mybir.AluOpType.mult
    at = fp16 if attn_fp16 else f32  # attention compute dtype
    st = fp16 if (attn_fp16 or attn_s16) else f32  # S-path dtype
    s_tree = attn_fp16 or attn_s16
    gt = f32r if gemm_f32r else f32  # gemm operand dtype

    ntiles = ntok // TILE_T

    nc = bacc.Bacc(
        "TRN2", target_bir_lowering=False, debug=False, num_devices=N_CORES
    )
    # x, host-packed so each partition's per-tile data is one contiguous 3KB
    # run: xH[p, tile, c, ti] = x[tile*128+ti, c*128+p]
    xH_d = nc.dram_tensor(
        "xH", [128, ntok // TILE_T, KC, TILE_T], f32, kind="ExternalInput"
    ).ap()
    waT_d = nc.dram_tensor("waT", [D, 3 * D], f32, kind="ExternalInput").ap()
    woT_d = nc.dram_tensor("woT", [D, D], f32, kind="ExternalInput").ap()
    nlam_d = nc.dram_tensor("nlam", [1, 1], f32, kind="ExternalInput").ap()
    out_d = nc.dram_tensor("out", [ntok, D], f32, kind="ExternalOutput").ap()

    with tile.TileContext(nc) as tc:
        from contextlib import ExitStack

        with ExitStack() as ctx:
            singles = ctx.enter_context(tc.tile_pool(name="singles", bufs=1))

            # ---- persistent weights / constants ----
            wdt = f32r if gemm_f32r else f32
            wa_sb = singles.tile([128, KC, 3 * D], wdt)
            wo_sb = singles.tile([128, KC, D], wdt)
            if gemm_f32r:
                # the BIR verifier requires every producer of an f32r matmul
                # operand to be a rounding op, so DMA into a temp pool and
                # round-copy into the persistent f32r tiles
                with tc.tile_pool(name="wtmp", bufs=1) as wtmp:
                    wa_raw = wtmp.tile([128, KC, 3 * D], f32, tag="wa_raw")
                    nc.sync.dma_start(
                        out=wa_raw[:], in_=waT_d.rearrange("(c p) o -> p c o", p=128)
                    )
                    nc.scalar.copy(wa_sb[:], wa_raw[:])
                    wo_raw = wtmp.tile([128, KC, D], f32, tag="wo_raw")
                    nc.sync.dma_start(
                        out=wo_raw[:], in_=woT_d.rearrange("(c p) o -> p c o", p=128)
                    )
                    nc.scalar.copy(wo_sb[:], wo_raw[:])
            else:
                nc.sync.dma_start(
                    out=wa_sb[:], in_=waT_d.rearrange("(c p) o -> p c o", p=128)
                )
                nc.sync.dma_start(
                    out=wo_sb[:], in_=woT_d.rearrange("(c p) o -> p c o", p=128)
                )
            ident = singles.tile([128, 128], f32)
            make_identity(nc, ident[:])
            nlam_sb = singles.tile([128, 1], f32)
            nc.gpsimd.dma_start(out=nlam_sb[:], in_=nlam_d.to_broadcast((128, 1)))

            xpool = ctx.enter_context(tc.tile_pool(name="xt", bufs=3))
            qkvpool = ctx.enter_context(tc.tile_pool(name="qkv", bufs=2))
            ppool = ctx.enter_context(tc.tile_pool(name="prod", bufs=1))
            popool = ctx.enter_context(
                tc.tile_pool(name="oprod", bufs=1 if attn_fp16 else 2)
            )
            smalls = ctx.enter_context(tc.tile_pool(name="smalls", bufs=2))
            ypool = ctx.enter_context(tc.tile_pool(name="y", bufs=2))
            opool = ctx.enter_context(tc.tile_pool(name="o2", bufs=2))
            ps_g1 = ctx.enter_context(tc.tile_pool(name="ps1", bufs=3, space="PSUM"))
            ps_tr = ctx.enter_context(tc.tile_pool(name="pstr", bufs=2, space="PSUM"))
            ps_g2 = ctx.enter_context(tc.tile_pool(name="ps2", bufs=2, space="PSUM"))

            def body():
                for it in range(ntiles):
                    emit_tile(it)

            def emit_tile(it):
                t0 = it * TILE_T
                # ---- load x^T tile (stationary chunks for GEMM1) ----
                xt_raw = xpool.tile([128, KC, TILE_T], f32, tag="xt_raw")
                nc.sync.dma_start(out=xt_raw[:], in_=xH_d[:, it, :, :])
                if gemm_f32r and "gemm1" in parts:
                    xt = xpool.tile([128, KC, TILE_T], f32r, tag="xt")
                    nc.scalar.copy(xt[:], xt_raw[:])
                else:
                    xt = xt_raw

                # ---- GEMM1: qkv[t, 0:2304] ----
                qkv = qkvpool.tile([128, 3 * D], at)
                if "gemm1" in parts:
                    for ob in range(5):
                        o0 = ob * 512
                        ow = min(512, 3 * D - o0)
                        ps = ps_g1.tile([128, 512], f32)
                        for c in range(KC):
                            nc.tensor.matmul(
                                ps[:, :ow],
                                xt[:, c, :],
                                wa_sb[:, c, o0 : o0 + ow],
                                start=(c == 0),
                                stop=(c == KC - 1),
                            )
                        nc.scalar.copy(qkv[:, o0 : o0 + ow], ps[:, :ow])
                else:
                    nc.scalar.copy(qkv[:, 0:768], xt[:].rearrange("p c t -> p (c t)"))
                    nc.scalar.copy(qkv[:, 768:1536], qkv[:, 0:768])
                    nc.scalar.copy(qkv[:, 1536:2304], qkv[:, 0:768])

                # ---- attention over heads, per token ----
                # The combine y = o1 - lam*o2 is linear in the softmax
                # weights, so fold it BEFORE the A.V contraction:
                #   y = sum_j (A1[j] - lam*A2[j]) * v[j]
                # -> only ONE (i,c,j)-product pass instead of two.
                y = ypool.tile([128, 768], f32, tag="y")
                if "attn" in parts:
                    if not attn_fp16:
                        # v^T view: [p, c(64), j(12)] contiguous inner j
                        vT = smalls.tile([128, 64, 12], f32, tag="vT")
                        nc.scalar.copy(
                            vT[:],
                            qkv[:, 2 * D : 3 * D].rearrange("p (j c) -> p c j", j=12),
                        )
                    if s_tree and not attn_fp16:
                        # cast q,k to fp16 so products run in the 2x tier
                        qk16 = smalls.tile([128, 2 * D], fp16, tag="qk16")
                        nc.scalar.copy(qk16[:], qkv[:, 0 : 2 * D])
                        qsrc, ksrc = qk16[:, 0:D], qk16[:, D : 2 * D]
                    else:
                        qsrc, ksrc = qkv[:, 0:D], qkv[:, D : 2 * D]
                    S_t = []
                    for s in (0, 1):
                        qv = qsrc.rearrange(
                            "p (i s d) -> p i s d", s=2, d=32
                        )[:, :, s, :]
                        kv = ksrc.rearrange(
                            "p (i s d) -> p i s d", s=2, d=32
                        )[:, :, s, :]
                        # products P[p, (i j), d] = q[p,i,d] * k[p,j,d]
                        P = ppool.tile([128, 144, 32], st, tag="P")
                        nc.vector.tensor_mul(
                            P[:].rearrange("p (i j) d -> p i j d", i=12),
                            qv.unsqueeze(2).broadcast_to((128, 12, 12, 32)),
                            kv.unsqueeze(1).broadcast_to((128, 12, 12, 32)),
                        )
                        S = smalls.tile([128, 144], st, tag=f"S{s}")
                        if s_tree:
                            # tree-sum over d (fp16 tensor_tensor runs 2x;
                            # tensor_reduce is always 1x)
                            T1 = smalls.tile([128, 144, 16], st, tag="T1")
                            nc.vector.tensor_add(T1[:], P[:, :, 0:16], P[:, :, 16:32])
                            T2 = smalls.tile([128, 144, 8], st, tag="T2")
                            nc.vector.tensor_add(T2[:], T1[:, :, 0:8], T1[:, :, 8:16])
                            T3 = smalls.tile([128, 144, 4], st, tag="T3")
                            nc.vector.tensor_add(T3[:], T2[:, :, 0:4], T2[:, :, 4:8])
                            T4 = smalls.tile([128, 144, 2], st, tag="T4")
                            nc.vector.tensor_add(T4[:], T3[:, :, 0:2], T3[:, :, 2:4])
                            nc.vector.tensor_add(S[:], T4[:, :, 0], T4[:, :, 1])
                        else:
                            nc.vector.tensor_reduce(
                                out=S[:], in_=P[:], axis=X, op=add
                            )
                        S_t.append(S)
                    # adjacent Exp activations (one ACT table switch per tile)
                    E_t = []
                    for s in (0, 1):
                        E = smalls.tile([128, 12, 12], at, tag=f"E{s}")
                        nc.scalar.activation(
                            out=E[:].rearrange("p i j -> p (i j)"),
                            in_=S_t[s][:],
                            func=Exp,
                            scale=SCALE,
                        )
                        E_t.append(E)
                    # Atilde = E1/den1 - lam * E2/den2
                    A_t = []
                    for s in (0, 1):
                        den = smalls.tile([128, 12], f32, tag=f"den{s}")
                        nc.vector.tensor_reduce(
                            out=den[:], in_=E_t[s][:], axis=X, op=add
                        )
                        rec = smalls.tile([128, 12], at, tag=f"rec{s}")
                        with nc.allow_low_precision(
                            reason="softmax weights tolerate reduced precision"
                        ):
                            nc.vector.reciprocal(out=rec[:], in_=den[:])
                        if s == 1:
                            # fold -lam into the sub-2 weights
                            nc.vector.tensor_scalar(
                                out=rec[:],
                                in0=rec[:],
                                scalar1=nlam_sb[:],
                                scalar2=None,
                                op0=mult,
                            )
                        A = smalls.tile([128, 12, 12], at, tag=f"A{s}")
                        nc.vector.tensor_mul(
                            A[:],
                            E_t[s][:],
                            rec[:].unsqueeze(2).broadcast_to((128, 12, 12)),
                        )
                        A_t.append(A)
                    At = smalls.tile([128, 12, 12], at, tag="At")
                    nc.vector.tensor_add(At[:], A_t[0][:], A_t[1][:])
                    # y[p, (i c)] = sum_j Atilde[p,i,j] * v[p,j,c]
                    if attn_fp16:
                        vv = qkv[:, 2 * D : 3 * D].rearrange("p (j c) -> p j c", j=12)
                        PO = popool.tile([128, 12, 768], at, tag="PO")
                        nc.vector.tensor_mul(
                            PO[:].rearrange("p j (i c) -> p j i c", i=12),
                            At[:]
                            .transpose([0, 2, 1])
                            .unsqueeze(3)
                            .broadcast_to((128, 12, 12, 64)),
                            vv.unsqueeze(2).broadcast_to((128, 12, 12, 64)),
                        )
                        U1 = popool.tile([128, 6, 768], at, tag="U1")
                        nc.vector.tensor_add(U1[:], PO[:, 0:6, :], PO[:, 6:12, :])
                        U2 = popool.tile([128, 3, 768], at, tag="U2")
                        nc.vector.tensor_add(U2[:], U1[:, 0:3, :], U1[:, 3:6, :])
                        U3 = popool.tile([128, 768], at, tag="U3")
                        nc.vector.tensor_add(U3[:], U2[:, 0, :], U2[:, 1, :])
                        nc.vector.tensor_add(y[:], U3[:], U2[:, 2, :])
                    else:
                        for h in (0, 1):
                            PO = popool.tile([128, 6, 64, 12], f32, tag="PO")
                            iA = At[:, h * 6 : (h + 1) * 6, :]
                            nc.vector.tensor_mul(
                                PO[:],
                                iA.unsqueeze(2).broadcast_to((128, 6, 64, 12)),
                                vT[:].unsqueeze(1).broadcast_to((128, 6, 64, 12)),
                            )
                            nc.vector.tensor_reduce(
                                out=y[:, h * 384 : (h + 1) * 384],
                                in_=PO[:].rearrange("p i c j -> p (i c) j"),
                                axis=X,
                                op=add,
                            )
                else:
                    nc.scalar.copy(y[:], qkv[:, 0:768])

                # ---- transpose y (PE) to feed GEMM2 stationary ----
                yT = ypool.tile([128, KC, 128], gt, tag="yT")
                if "tr" in parts:
                    for c in range(KC):
                        tp = ps_tr.tile([128, 128], f32)
                        nc.tensor.transpose(
                            tp[:], y[:, c * 128 : (c + 1) * 128], ident[:]
                        )
                        nc.scalar.copy(yT[:, c, :], tp[:])
                else:
                    nc.scalar.copy(
                        yT[:], y[:].rearrange("p (c t) -> p c t", c=KC)
                    )

                # ---- GEMM2: out2 = y @ woT ----
                o2 = opool.tile([128, D], f32)
                if "gemm2" in parts:
                    for o0, ow in ((0, 512), (512, 256)):
                        ps2 = ps_g2.tile([128, 512], f32)
                        for c in range(KC):
                            nc.tensor.matmul(
                                ps2[:, :ow],
                                yT[:, c, :],
                                wo_sb[:, c, o0 : o0 + ow],
                                start=(c == 0),
                                stop=(c == KC - 1),
                            )
                        nc.scalar.copy(o2[:, o0 : o0 + ow], ps2[:, :ow])
                else:
                    nc.scalar.copy(
                        o2[:], yT[:].rearrange("p c t -> p (c t)")
                    )
                nc.sync.dma_start(out=out_d[t0 : t0 + TILE_T, :], in_=o2[:])

            if nrep == 1:
                body()
            else:
                with tc.For_i(0, nrep, 1):
                    body()

    nc.compile()
    return nc


# chosen configuration (HW-measured, steady-state repeat-loop delta, per
# full per-core pass = whole problem since the 8 cores run in parallel):
#   f32 GEMM + f32 attn  : ~2.5 ms, rel err 7.6e-6
#   f32r GEMM + f32 attn : ~1.3-1.45 ms, rel err 2.5e-4   <- default
#   f32r GEMM + s16 attn : ~1.14 ms, rel err 7.5e-4  (ATTN_S16: fp16 S-path)
#   f32r GEMM + fp16 attn: ~1.0-1.1 ms, rel err ~1.5e-3 (ATTN_FP16: full fp16)
# f32r (TF32-like single-pass PE mode, 4x f32 matmul throughput) keeps the
# absolute error at 2.1e-5 vs output absmax 8.5e-2 — well inside a 1e-3
# scale-relative gate; flip GEMM_F32R off for bit-conservative f32.
GEMM_F32R = True
ATTN_FP16 = False
ATTN_S16 = False
GPSIMD_SPLIT = False  # measured net-negative (DVE<->GpSimd SBUF port contention)


def _get_program(ntok=TOK_PER_CORE, nrep=1):
    key = (ntok, nrep, GEMM_F32R, ATTN_FP16, ATTN_S16, GPSIMD_SPLIT)
    if key not in _PROG_CACHE:
        _PROG_CACHE[key] = _build_program(
            ntok, nrep, gemm_f32r=GEMM_F32R, attn_fp16=ATTN_FP16,
            attn_s16=ATTN_S16, gpsimd_split=GPSIMD_SPLIT,
        )
    return _PROG_CACHE[key]


def kernel(x, W_attn, W_out, lambda_q1, lambda_k1, lambda_q2, lambda_k2):
    x = np.asarray(x, dtype=np.float32)
    W_attn = np.asarray(W_attn, dtype=np.float32)
    W_out = np.asarray(W_out, dtype=np.float32)
    lambda_q1 = np.asarray(lambda_q1, dtype=np.float32)
    lambda_k1 = np.asarray(lambda_k1, dtype=np.float32)
    lambda_q2 = np.asarray(lambda_q2, dtype=np.float32)
    lambda_k2 = np.asarray(lambda_k2, dtype=np.float32)

    lam1 = np.exp(np.sum(lambda_q1 * lambda_k1))
    lam2 = np.exp(np.sum(lambda_q2 * lambda_k2))
    lam_full = np.float32(lam1 - lam2 + np.float32(LAMBDA_INIT))
    nlam = np.array([[-lam_full]], dtype=np.float32)

    waT = np.ascontiguousarray(W_attn.T)  # (768, 2304)
    woT = np.ascontiguousarray(W_out.T) * np.float32(1.0 - LAMBDA_INIT)
    woT = np.ascontiguousarray(woT, dtype=np.float32)

    xf = x.reshape(TOK_TOTAL, D)
    ntiles = TOK_PER_CORE // TILE_T
    in_maps = []
    for c in range(N_CORES):
        xs = xf[c * TOK_PER_CORE : (c + 1) * TOK_PER_CORE]
        # xH[p, tile, c, ti] = xs[tile*128+ti, c*128+p]
        xh = np.ascontiguousarray(
            xs.reshape(ntiles, TILE_T, KC, 128).transpose(3, 0, 2, 1)
        )
        in_maps.append({"xH": xh, "waT": waT, "woT": woT, "nlam": nlam})

    from concourse.bass_utils import run_bass_kernel_spmd

    nc = _get_program()
    res = run_bass_kernel_spmd(nc, in_maps, list(range(N_CORES)))
    outs = [res.results[i]["out"] for i in range(N_CORES)]
    y = np.concatenate(outs, axis=0).reshape(B, N, D)
    return y
